# revision 2
# baseline (speedup 1.0000x reference)
"""GGNN (JITGNN) Trainium2 kernel v2: 8-core row-parallel SpMM message passing.

Strategy (per sharding hint): shard the [N+1, N+1] adjacency row-wise across
8 cores. Each core keeps the h-state for its 1000(+1) nodes in transposed
(feature-major) layout in SBUF, computes its slice of messages each timestep,
AllGathers fp16 messages across cores, then streams its pre-transposed
adjacency shard as the matmul moving operand to aggregate, and applies the
GRU cell to its slice. Two independent graphs (b, a) are interleaved so each
graph's collective hides behind the other graph's compute.

Numerics: fp16 matmul operands (messages, adjacency, gate weights) with fp32
PSUM accumulation and fp32 state/elementwise. The final output depends only
on the supernode row, which sums ~8000 messages; the supernode gets an exact
path: each core computes its fp32 message column-sum (via fp32 row-reduced h
and an fp32 matmul), transmits it as an fp16 hi+lo pair through the same
AllGather (rows 1000/1001 of its shard block, which are otherwise padding),
and the supernode's GRU lane is recomputed in fp32. Final 2-class head on
host in fp64.
"""

import numpy as np

try:
    import concourse.bacc  # noqa: F401
except ImportError:  # pragma: no cover
    import sys

    sys.path.insert(0, "/opt/trn_rl_repo")

F16 = np.float16
HIDDEN = 256
N = 8000
NC = 8             # cores
SLOT = 1024        # padded node slots per core (1000 real, +1 supernode on core 7)
REAL = N // NC     # 1000 real rows per core
JTOT = NC * SLOT   # 8192 padded message rows
NKT = JTOT // 128  # 64 contraction k-tiles
ACH = 8            # A chunks per graph, each [128, 8192] = 8 k-tiles x 1024 cols


def _prep_adj_shards(adj):
    """adj [8000,8000] 0/1 fp32 -> per-core rhs chunks [ACH, 128, 8192] fp16.

    R_c[j', u] = A_aug[i(u), j(j')] with j' = 1024*d + r (msgs row layout of
    the AllGather output), u = local output slot. The supernode column
    (u=1000 on core 7) sums the per-core fp32 colsum hi/lo rows at
    j' = 1024*d + {1000, 1001} instead of the raw message rows.
    """
    AT = np.ascontiguousarray(adj.T.astype(F16))             # [j, i]
    ATj = np.zeros((JTOT, N), dtype=F16)
    for d in range(NC):
        ATj[SLOT * d : SLOT * d + REAL] = AT[REAL * d : REAL * (d + 1)]
    supersum = np.zeros((JTOT,), dtype=F16)
    for d in range(NC):
        supersum[SLOT * d + REAL] = 1.0      # colsum hi row
        supersum[SLOT * d + REAL + 1] = 1.0  # colsum lo row
    shards = []
    for c in range(NC):
        R = np.zeros((JTOT, SLOT), dtype=F16)
        R[:, :REAL] = ATj[:, REAL * c : REAL * (c + 1)]
        if c == NC - 1:
            R[:, REAL] = supersum
        chunks = R.reshape(ACH, 8, 128, SLOT).transpose(0, 2, 1, 3).reshape(ACH, 128, 8 * SLOT)
        shards.append(np.ascontiguousarray(chunks))
    return shards


def _prep_h0_shards(x):
    """x [8000, 256] fp32 -> per-core transposed state [2, 128, SLOT] fp32."""
    xT = x.T.astype(np.float32)  # [256, 8000]
    shards = []
    for c in range(NC):
        H = np.zeros((HIDDEN, SLOT), dtype=np.float32)
        H[:, :REAL] = xT[:, REAL * c : REAL * (c + 1)]
        shards.append(np.ascontiguousarray(H.reshape(2, 128, SLOT)))
    return shards


def _pack_lhsT(w_t, cols, dt):
    """w_t [256, cols] -> packed [128, 2*cols] with free = kt*cols + c."""
    return np.ascontiguousarray(
        w_t.astype(dt).reshape(2, 128, cols).transpose(1, 0, 2).reshape(128, 2 * cols)
    )


def _build_program(T):
    import concourse.bacc as bacc
    import concourse.mybir as mybir
    from concourse import tile

    f16 = mybir.dt.float16
    f32 = mybir.dt.float32
    Alu = mybir.AluOpType
    Act = mybir.ActivationFunctionType
    Ax = mybir.AxisListType

    nc = bacc.Bacc("TRN2", target_bir_lowering=False, debug=False, num_devices=NC)

    GR = ("b", "a")
    A_in = {g: nc.dram_tensor(f"A_{g}", [ACH, 128, 8 * SLOT], f16, kind="ExternalInput") for g in GR}
    H0_in = {g: nc.dram_tensor(f"h0_{g}", [2, 128, SLOT], f32, kind="ExternalInput") for g in GR}
    Wlin32_in = nc.dram_tensor("Wlin32", [128, 512], f32, kind="ExternalInput")
    Wlin16_in = nc.dram_tensor("Wlin16", [128, 512], f16, kind="ExternalInput")
    Wih16_in = nc.dram_tensor("Wih16", [128, 1536], f16, kind="ExternalInput")
    Whh16_in = nc.dram_tensor("Whh16", [128, 1536], f16, kind="ExternalInput")
    Wih32_in = nc.dram_tensor("Wih32", [128, 1536], f32, kind="ExternalInput")
    Whh32_in = nc.dram_tensor("Whh32", [128, 1536], f32, kind="ExternalInput")
    Blin_in = nc.dram_tensor("Blin", [128, 256], f32, kind="ExternalInput")
    Bcol_in = nc.dram_tensor("Bcol", [2, 128], f32, kind="ExternalInput")  # 1000*b_lin, feature-major
    Brz_in = nc.dram_tensor("Brz", [4, 128], f32, kind="ExternalInput")
    Bin_in = nc.dram_tensor("Bin", [2, 128], f32, kind="ExternalInput")
    Bhn_in = nc.dram_tensor("Bhn", [2, 128], f32, kind="ExternalInput")
    HO_out = {g: nc.dram_tensor(f"ho_{g}", [2, 128, SLOT], f32, kind="ExternalOutput") for g in GR}

    rg = [list(range(NC))]

    with tile.TileContext(nc) as tc:
        with (
            tc.tile_pool(name="const", bufs=1) as constp,
            tc.tile_pool(name="a_stream", bufs=2) as a_pool,
            tc.tile_pool(name="lhs_stream", bufs=4) as lhs_pool,
            tc.tile_pool(name="state", bufs=2) as state_pool,
            tc.tile_pool(name="state16", bufs=2) as state16_pool,
            tc.tile_pool(name="work", bufs=1) as work_pool,
            tc.tile_pool(name="tmp", bufs=2) as tmp_pool,
            tc.tile_pool(name="micro", bufs=2) as micro_pool,
            tc.tile_pool(name="psA", bufs=2, space="PSUM") as psum_agg,
            tc.tile_pool(name="psG", bufs=2, space="PSUM") as psum_gates,
            tc.tile_pool(name="dram", bufs=2, space="DRAM") as dram_pool,
        ):
            # ---- constants ----
            wlin32 = constp.tile([128, 512], f32, name="wlin32")
            nc.sync.dma_start(wlin32[:], Wlin32_in[:])
            wlin16 = constp.tile([128, 512], f16, name="wlin16")
            nc.sync.dma_start(wlin16[:], Wlin16_in[:])
            wih16 = constp.tile([128, 1536], f16, name="wih16")
            nc.sync.dma_start(wih16[:], Wih16_in[:])
            whh16 = constp.tile([128, 1536], f16, name="whh16")
            nc.sync.dma_start(whh16[:], Whh16_in[:])
            wih32 = constp.tile([128, 1536], f32, name="wih32")
            nc.sync.dma_start(wih32[:], Wih32_in[:])
            whh32 = constp.tile([128, 1536], f32, name="whh32")
            nc.sync.dma_start(whh32[:], Whh32_in[:])
            blin = constp.tile([128, 256], f32, name="blin")
            nc.sync.dma_start(blin[:], Blin_in[:])
            bcol = constp.tile([128, 2], f32, name="bcol")
            brz = constp.tile([128, 4], f32, name="brz")
            for i in range(4):
                nc.sync.dma_start(brz[:, i : i + 1], Brz_in[i : i + 1, :].rearrange("o p -> p o"))
            bin_ = constp.tile([128, 2], f32, name="bin_")
            bhn = constp.tile([128, 2], f32, name="bhn")
            for i in range(2):
                nc.sync.dma_start(bcol[:, i : i + 1], Bcol_in[i : i + 1, :].rearrange("o p -> p o"))
                nc.sync.dma_start(bin_[:, i : i + 1], Bin_in[i : i + 1, :].rearrange("o p -> p o"))
                nc.sync.dma_start(bhn[:, i : i + 1], Bhn_in[i : i + 1, :].rearrange("o p -> p o"))

            # ---- state load ----
            H = {}
            H16 = {}
            for g in GR:
                H[g] = []
                H16[g] = []
                for i in range(2):
                    h = state_pool.tile([128, SLOT], f32, name=f"h_{g}{i}", tag=f"h_{g}{i}")
                    nc.sync.dma_start(h[:], H0_in[g][i, :, :])
                    H[g].append(h)
                    h16 = state16_pool.tile([128, SLOT], f16, name=f"h16_{g}{i}", tag=f"h16_{g}{i}")
                    nc.vector.tensor_copy(h16[:], h[:])
                    H16[g].append(h16)

            cc_out = {}

            def emit_msgs_allgather(g):
                """fp16 msgs slice + fp32 colsum hi/lo -> cc_in -> AllGather."""
                msgs = work_pool.tile([128, 8 * 256], f16, name=f"msgs_{g}", tag=f"msgs_{g}")
                for mi in range(8):
                    ps = psum_gates.tile([128, 1024], f32, name=f"psm_{g}{mi}", tag="psG")
                    for kt in range(2):
                        nc.tensor.matmul(
                            ps[:, 0:256],
                            lhsT=H16[g][kt][:, mi * 128 : (mi + 1) * 128],
                            rhs=wlin16[:, kt * 256 : (kt + 1) * 256],
                            start=(kt == 0),
                            stop=(kt == 1),
                        )
                    nc.vector.tensor_add(msgs[:, mi * 256 : (mi + 1) * 256], ps[:, 0:256], blin[:])
                # exact supernode contribution: fp32 colsum of this core's msgs
                ps_cs = psum_gates.tile([128, 1024], f32, name=f"pscs_{g}", tag="psG")
                hs = []
                for kt in range(2):
                    hst = micro_pool.tile([128, 1], f32, name=f"hs_{g}{kt}", tag=f"hs_{g}{kt}")
                    nc.vector.tensor_reduce(hst[:], H[g][kt][:, 0:REAL], Ax.X, Alu.add)
                    hs.append(hst)
                for mi in range(2):
                    for kt in range(2):
                        nc.tensor.matmul(
                            ps_cs[:, mi : mi + 1],
                            lhsT=wlin32[:, kt * 256 + mi * 128 : kt * 256 + (mi + 1) * 128],
                            rhs=hs[kt][:],
                            start=(kt == 0),
                            stop=(kt == 1),
                        )
                cs_hi, cs_lo = [], []
                for mi in range(2):
                    cs = micro_pool.tile([128, 1], f32, name=f"cs_{g}{mi}", tag=f"cs_{g}{mi}")
                    nc.vector.tensor_scalar_add(cs[:], ps_cs[:, mi : mi + 1], bcol[:, mi : mi + 1])
                    hi = micro_pool.tile([128, 1], f16, name=f"cshi_{g}{mi}", tag=f"cshi_{g}{mi}")
                    nc.vector.tensor_copy(hi[:], cs[:])
                    lo = micro_pool.tile([128, 1], f16, name=f"cslo_{g}{mi}", tag=f"cslo_{g}{mi}")
                    nc.vector.tensor_sub(lo[:], cs[:], hi[:])
                    cs_hi.append(hi)
                    cs_lo.append(lo)
                cc_in = dram_pool.tile([SLOT, 256], f16, name=f"cc_in_{g}", tag=f"cc_in_{g}")
                nc.sync.dma_start(
                    cc_in[:].rearrange("(a p) f -> p a f", p=128),
                    msgs[:].rearrange("p (a f) -> p a f", a=8),
                )
                for mi in range(2):
                    nc.sync.dma_start(
                        cc_in[REAL : REAL + 1, mi * 128 : (mi + 1) * 128].rearrange("o f -> f o"),
                        cs_hi[mi][:],
                    )
                    nc.sync.dma_start(
                        cc_in[REAL + 1 : REAL + 2, mi * 128 : (mi + 1) * 128].rearrange("o f -> f o"),
                        cs_lo[mi][:],
                    )
                cco = dram_pool.tile(
                    [JTOT, 256], f16, name=f"cc_out_{g}", tag=f"cc_out_{g}", addr_space="Shared"
                )
                nc.gpsimd.collective_compute(
                    "AllGather",
                    mybir.AluOpType.bypass,
                    replica_groups=rg,
                    ins=[cc_in.opt()],
                    outs=[cco.opt()],
                )
                cc_out[g] = cco

            def emit_agg(g):
                """m.T [256, SLOT] = msgs_full.T @ A_shard.T via 64 k-tiles."""
                psA = [
                    psum_agg.tile([128, SLOT], f32, name=f"psA_{g}{mi}", tag="psA")
                    for mi in range(2)
                ]
                lhs_tiles = {}
                for q in range(NKT // 4):  # 16 lhs loads of 4 k-tiles each
                    off = 512 * q
                    lt = lhs_pool.tile([128, 1024], f16, name=f"lhs_{g}{q}", tag="lhs")
                    nc.sync.dma_start(
                        lt[:].rearrange("p (a f) -> p a f", a=4),
                        cc_out[g][off : off + 512, :].rearrange("(a p) f -> p a f", p=128),
                    )
                    lhs_tiles[q] = lt
                for g8 in range(ACH):
                    at = a_pool.tile([128, 8 * SLOT], f16, name=f"at_{g}{g8}", tag="at")
                    nc.sync.dma_start(at[:], A_in[g][g8, :, :])
                    for ktl in range(8):
                        kt = g8 * 8 + ktl
                        lt = lhs_tiles[kt // 4]
                        lo = (kt % 4) * 256
                        for mi in range(2):
                            for ni in range(2):
                                rhs_sl = at[:, ktl * SLOT + ni * 512 : ktl * SLOT + (ni + 1) * 512]
                                nc.tensor.matmul(
                                    psA[mi][:, ni * 512 : (ni + 1) * 512],
                                    lhsT=lt[:, lo + mi * 128 : lo + (mi + 1) * 128],
                                    rhs=rhs_sl,
                                    start=(kt == 0),
                                    stop=(kt == NKT - 1),
                                )
                m16 = []
                m_sup = []
                for mi in range(2):
                    msup = micro_pool.tile([128, 1], f32, name=f"msup_{g}{mi}", tag=f"msup_{g}{mi}")
                    nc.vector.tensor_copy(msup[:], psA[mi][:, REAL : REAL + 1])
                    m_sup.append(msup)
                    mt = work_pool.tile([128, SLOT], f16, name=f"m16_{g}{mi}", tag=f"m16_{g}{mi}")
                    nc.vector.tensor_copy(mt[:], psA[mi][:])
                    m16.append(mt)
                return m16, m_sup

            def emit_gru(g, m16, m_sup):
                """Gate matmuls (fp16) + fp32 elementwise GRU update of H[g]/H16[g].

                The supernode lane (column REAL) is recomputed in fp32 and
                overwritten into the new state tiles.
                """
                old_H = list(H[g])
                old_H16 = list(H16[g])

                def gate_psum(G, name):
                    ps = psum_gates.tile([128, 1024], f32, name=name, tag="psG")
                    for ni in range(2):
                        n_mm = 0
                        for kt in range(2):
                            for w, r in ((wih16, m16), (whh16, old_H16)):
                                nc.tensor.matmul(
                                    ps[:, ni * 512 : (ni + 1) * 512],
                                    lhsT=w[:, kt * 768 + G * 128 : kt * 768 + (G + 1) * 128],
                                    rhs=r[kt][:, ni * 512 : (ni + 1) * 512],
                                    start=(n_mm == 0),
                                    stop=(n_mm == 3),
                                )
                                n_mm += 1
                    return ps

                def half_psum(G, w, r, name):
                    ps = psum_gates.tile([128, 1024], f32, name=name, tag="psG")
                    for ni in range(2):
                        for kt in range(2):
                            nc.tensor.matmul(
                                ps[:, ni * 512 : (ni + 1) * 512],
                                lhsT=w[:, kt * 768 + G * 128 : kt * 768 + (G + 1) * 128],
                                rhs=r[kt][:, ni * 512 : (ni + 1) * 512],
                                start=(kt == 0),
                                stop=(kt == 1),
                            )
                    return ps

                # fp32 supernode gate psums: one psG slot, 8 columns
                # cols 0..3 = r0,r1,z0,z1 (gi+gh); 4,5 = inn0,inn1; 6,7 = hn0,hn1
                ps_s = psum_gates.tile([128, 1024], f32, name=f"ps_s{g}", tag="psG")
                h_sup = [old_H[kt][:, REAL : REAL + 1] for kt in range(2)]
                for G in range(4):
                    n_mm = 0
                    for kt in range(2):
                        for w, r in ((wih32, m_sup), (whh32, h_sup)):
                            nc.tensor.matmul(
                                ps_s[:, G : G + 1],
                                lhsT=w[:, kt * 768 + G * 128 : kt * 768 + (G + 1) * 128],
                                rhs=r[kt],
                                start=(n_mm == 0),
                                stop=(n_mm == 3),
                            )
                            n_mm += 1
                for ch in range(2):
                    for col, w, r in ((4 + ch, wih32, m_sup), (6 + ch, whh32, h_sup)):
                        for kt in range(2):
                            nc.tensor.matmul(
                                ps_s[:, col : col + 1],
                                lhsT=w[:, kt * 768 + (4 + ch) * 128 : kt * 768 + (5 + ch) * 128],
                                rhs=r[kt],
                                start=(kt == 0),
                                stop=(kt == 1),
                            )

                rr, zz = [], []
                for ch in range(2):
                    ps = gate_psum(ch, f"ps_r{g}{ch}")
                    r_t = work_pool.tile([128, SLOT], f32, name=f"r_{g}{ch}", tag=f"r_{g}{ch}")
                    nc.scalar.activation(r_t[:], ps[:], Act.Sigmoid, bias=brz[:, ch : ch + 1])
                    rr.append(r_t)
                for ch in range(2):
                    ps = gate_psum(2 + ch, f"ps_z{g}{ch}")
                    z_t = work_pool.tile([128, SLOT], f32, name=f"z_{g}{ch}", tag=f"z_{g}{ch}")
                    nc.scalar.activation(z_t[:], ps[:], Act.Sigmoid, bias=brz[:, 2 + ch : 3 + ch])
                    zz.append(z_t)

                # supernode fp32 lane: r/z/n + update into [128,1] tiles
                sup_new = []
                for ch in range(2):
                    rs = micro_pool.tile([128, 1], f32, name=f"rs_{g}{ch}", tag=f"rs_{g}{ch}")
                    nc.scalar.activation(rs[:], ps_s[:, ch : ch + 1], Act.Sigmoid, bias=brz[:, ch : ch + 1])
                    zs = micro_pool.tile([128, 1], f32, name=f"zs_{g}{ch}", tag=f"zs_{g}{ch}")
                    nc.scalar.activation(zs[:], ps_s[:, 2 + ch : 3 + ch], Act.Sigmoid, bias=brz[:, 2 + ch : 3 + ch])
                    t1s = micro_pool.tile([128, 1], f32, name=f"t1s_{g}{ch}", tag=f"t1s_{g}{ch}")
                    nc.vector.scalar_tensor_tensor(
                        t1s[:], ps_s[:, 6 + ch : 7 + ch], bhn[:, ch : ch + 1], rs[:], Alu.add, Alu.mult
                    )
                    t2s = micro_pool.tile([128, 1], f32, name=f"t2s_{g}{ch}", tag=f"t2s_{g}{ch}")
                    nc.vector.tensor_add(t2s[:], t1s[:], ps_s[:, 4 + ch : 5 + ch])
                    ns = micro_pool.tile([128, 1], f32, name=f"ns_{g}{ch}", tag=f"ns_{g}{ch}")
                    nc.scalar.activation(ns[:], t2s[:], Act.Tanh, bias=bin_[:, ch : ch + 1])
                    ds = micro_pool.tile([128, 1], f32, name=f"ds_{g}{ch}", tag=f"ds_{g}{ch}")
                    nc.vector.tensor_sub(ds[:], h_sup[ch], ns[:])
                    t3s = micro_pool.tile([128, 1], f32, name=f"t3s_{g}{ch}", tag=f"t3s_{g}{ch}")
                    nc.vector.tensor_mul(t3s[:], zs[:], ds[:])
                    hns = micro_pool.tile([128, 1], f32, name=f"hns_{g}{ch}", tag=f"hns_{g}{ch}")
                    nc.vector.tensor_add(hns[:], ns[:], t3s[:])
                    sup_new.append(hns)

                for ch in range(2):
                    ps_i = half_psum(4 + ch, wih16, m16, f"ps_i{g}{ch}")
                    ps_h = half_psum(4 + ch, whh16, old_H16, f"ps_h{g}{ch}")
                    t1 = tmp_pool.tile([128, SLOT], f32, name=f"t1_{g}{ch}", tag=f"tmp_{g}")
                    nc.vector.scalar_tensor_tensor(
                        t1[:], ps_h[:], bhn[:, ch : ch + 1], rr[ch][:], Alu.add, Alu.mult
                    )
                    t2 = tmp_pool.tile([128, SLOT], f32, name=f"t2_{g}{ch}", tag=f"tmp_{g}")
                    nc.vector.tensor_add(t2[:], t1[:], ps_i[:])
                    n_t = tmp_pool.tile([128, SLOT], f32, name=f"n_{g}{ch}", tag=f"n_{g}")
                    nc.scalar.activation(n_t[:], t2[:], Act.Tanh, bias=bin_[:, ch : ch + 1])
                    d_t = tmp_pool.tile([128, SLOT], f32, name=f"d_{g}{ch}", tag=f"tmp_{g}")
                    nc.vector.tensor_sub(d_t[:], old_H[ch][:], n_t[:])
                    t3 = tmp_pool.tile([128, SLOT], f32, name=f"t3_{g}{ch}", tag=f"tmp_{g}")
                    nc.vector.tensor_mul(t3[:], zz[ch][:], d_t[:])
                    hn_new = state_pool.tile([128, SLOT], f32, name=f"h_{g}{ch}", tag=f"h_{g}{ch}")
                    nc.vector.tensor_add(hn_new[:], n_t[:], t3[:])
                    # overwrite supernode lane with the fp32 value
                    nc.vector.tensor_copy(hn_new[:, REAL : REAL + 1], sup_new[ch][:])
                    H[g][ch] = hn_new
                    h16_new = state16_pool.tile(
                        [128, SLOT], f16, name=f"h16_{g}{ch}", tag=f"h16_{g}{ch}"
                    )
                    nc.vector.tensor_copy(h16_new[:], hn_new[:])
                    H16[g][ch] = h16_new

            if T >= 1:
                for g in GR:
                    emit_msgs_allgather(g)
                for t in range(T):
                    for g in GR:
                        m16, m_sup = emit_agg(g)
                        emit_gru(g, m16, m_sup)
                        if t < T - 1:
                            emit_msgs_allgather(g)

            for g in GR:
                for i in range(2):
                    nc.sync.dma_start(HO_out[g][i, :, :], H[g][i][:])

    nc.compile()
    return nc


def prepare(inputs):
    """Build+compile the program and the per-core input maps.

    Returns (nc, in_maps, postprocess) where postprocess maps core 7's
    result dict to the final [2] log-softmax output.
    """
    b_x = np.asarray(inputs["b_x"], dtype=np.float32)
    a_x = np.asarray(inputs["a_x"], dtype=np.float32)
    b_adj = np.asarray(inputs["b_adj"], dtype=np.float32)
    a_adj = np.asarray(inputs["a_adj"], dtype=np.float32)
    W_lin = np.asarray(inputs["W_lin"], dtype=np.float32)
    b_lin = np.asarray(inputs["b_lin"], dtype=np.float32)
    W_ih = np.asarray(inputs["W_ih"], dtype=np.float32)
    b_ih = np.asarray(inputs["b_ih"], dtype=np.float32)
    W_hh = np.asarray(inputs["W_hh"], dtype=np.float32)
    b_hh = np.asarray(inputs["b_hh"], dtype=np.float32)
    W_fc = np.asarray(inputs["W_fc"], dtype=np.float32)
    b_fc = np.asarray(inputs["b_fc"], dtype=np.float32)
    T = int(inputs["n_timesteps"])

    nc = _build_program(T)

    A_shards = {"b": _prep_adj_shards(b_adj), "a": _prep_adj_shards(a_adj)}
    H0_shards = {"b": _prep_h0_shards(b_x), "a": _prep_h0_shards(a_x)}
    wlin32_p = _pack_lhsT(W_lin.T, 256, np.float32)
    wlin16_p = _pack_lhsT(W_lin.T, 256, np.float16)
    wih16_p = _pack_lhsT(W_ih.T, 768, np.float16)
    whh16_p = _pack_lhsT(W_hh.T, 768, np.float16)
    wih32_p = _pack_lhsT(W_ih.T, 768, np.float32)
    whh32_p = _pack_lhsT(W_hh.T, 768, np.float32)
    blin_b = np.ascontiguousarray(np.broadcast_to(b_lin.astype(np.float32), (128, 256)))
    bcol_p = np.ascontiguousarray((float(REAL) * b_lin).astype(np.float32).reshape(2, 128))
    brz_p = np.ascontiguousarray((b_ih[:512] + b_hh[:512]).astype(np.float32).reshape(4, 128))
    bin_p = np.ascontiguousarray(b_ih[512:768].astype(np.float32).reshape(2, 128))
    bhn_p = np.ascontiguousarray(b_hh[512:768].astype(np.float32).reshape(2, 128))

    in_maps = []
    for c in range(NC):
        in_maps.append(
            {
                "A_b": A_shards["b"][c],
                "A_a": A_shards["a"][c],
                "h0_b": H0_shards["b"][c],
                "h0_a": H0_shards["a"][c],
                "Wlin32": wlin32_p,
                "Wlin16": wlin16_p,
                "Wih16": wih16_p,
                "Whh16": whh16_p,
                "Wih32": wih32_p,
                "Whh32": whh32_p,
                "Blin": blin_b,
                "Bcol": bcol_p,
                "Brz": brz_p,
                "Bin": bin_p,
                "Bhn": bhn_p,
            }
        )

    def post(out7):
        sup = np.concatenate(
            [
                np.asarray(out7["ho_b"]).reshape(HIDDEN, SLOT)[:, REAL],
                np.asarray(out7["ho_a"]).reshape(HIDDEN, SLOT)[:, REAL],
            ]
        ).astype(np.float64)
        logits = sup @ W_fc.astype(np.float64).T + b_fc.astype(np.float64)
        mx = logits.max()
        return (logits - mx - np.log(np.exp(logits - mx).sum())).astype(np.float32)

    return nc, in_maps, post


def run(inputs, trace=False):
    from concourse.bass_utils import run_bass_kernel_spmd

    nc, in_maps, post = prepare(inputs)
    res = run_bass_kernel_spmd(nc, in_maps, core_ids=list(range(NC)), trace=trace)
    return post(res.results[NC - 1]), res.exec_time_ns


def kernel(**inputs):
    out, _ = run(inputs, trace=False)
    return out


# revision 3
# speedup vs baseline: 1.3213x; 1.3213x over previous
"""GGNN (JITGNN) Trainium2 kernel v2: 8-core row-parallel SpMM message passing.

Strategy (per sharding hint): shard the [N+1, N+1] adjacency row-wise across
8 cores. Each core keeps the h-state for its 1000(+1) nodes in transposed
(feature-major) layout in SBUF, computes its slice of messages each timestep,
AllGathers fp16 messages across cores, then streams its pre-transposed
adjacency shard as the matmul moving operand to aggregate, and applies the
GRU cell to its slice. Two independent graphs (b, a) are interleaved so each
graph's collective hides behind the other graph's compute.

Numerics: fp16 matmul operands (messages, adjacency, gate weights) with fp32
PSUM accumulation and fp32 state/elementwise. The final output depends only
on the supernode row, which sums ~8000 messages; the supernode gets an exact
path: each core computes its fp32 message column-sum (via fp32 row-reduced h
and an fp32 matmul), transmits it as an fp16 hi+lo pair through the same
AllGather (rows 1000/1001 of its shard block, which are otherwise padding),
and the supernode's GRU lane is recomputed in fp32. Final 2-class head on
host in fp64.
"""

import numpy as np

try:
    import concourse.bacc  # noqa: F401
except ImportError:  # pragma: no cover
    import sys

    sys.path.insert(0, "/opt/trn_rl_repo")

F16 = np.float16
HIDDEN = 256
N = 8000
NC = 8             # cores
SLOT = 1024        # padded node slots per core (1000 real, +1 supernode on core 7)
REAL = N // NC     # 1000 real rows per core
JTOT = NC * SLOT   # 8192 padded message rows
NKT = JTOT // 128  # 64 contraction k-tiles
ACH = 8            # A chunks per graph, each [128, 8192] = 8 k-tiles x 1024 cols


def _prep_adj_shards(adj):
    """adj [8000,8000] 0/1 fp32 -> per-core rhs chunks [ACH, 128, 8192] fp16.

    R_c[j', u] = A_aug[i(u), j(j')] with j' = 1024*d + r (msgs row layout of
    the AllGather output), u = local output slot. The supernode column
    (u=1000 on core 7) sums the per-core fp32 colsum hi/lo rows at
    j' = 1024*d + {1000, 1001} instead of the raw message rows.
    """
    AT = np.ascontiguousarray(adj.T.astype(F16))             # [j, i]
    ATj = np.zeros((JTOT, N), dtype=F16)
    for d in range(NC):
        ATj[SLOT * d : SLOT * d + REAL] = AT[REAL * d : REAL * (d + 1)]
    supersum = np.zeros((JTOT,), dtype=F16)
    for d in range(NC):
        supersum[SLOT * d + REAL] = 1.0      # colsum hi row
        supersum[SLOT * d + REAL + 1] = 1.0  # colsum lo row
    shards = []
    for c in range(NC):
        R = np.zeros((JTOT, SLOT), dtype=F16)
        R[:, :REAL] = ATj[:, REAL * c : REAL * (c + 1)]
        if c == NC - 1:
            R[:, REAL] = supersum
        chunks = R.reshape(ACH, 8, 128, SLOT).transpose(0, 2, 1, 3).reshape(ACH, 128, 8 * SLOT)
        shards.append(np.ascontiguousarray(chunks))
    return shards


def _prep_h0_shards(x):
    """x [8000, 256] fp32 -> per-core transposed state [2, 128, SLOT] fp32."""
    xT = x.T.astype(np.float32)  # [256, 8000]
    shards = []
    for c in range(NC):
        H = np.zeros((HIDDEN, SLOT), dtype=np.float32)
        H[:, :REAL] = xT[:, REAL * c : REAL * (c + 1)]
        shards.append(np.ascontiguousarray(H.reshape(2, 128, SLOT)))
    return shards


def _pack_lhsT(w_t, cols, dt):
    """w_t [256, cols] -> packed [128, 2*cols] with free = kt*cols + c."""
    return np.ascontiguousarray(
        w_t.astype(dt).reshape(2, 128, cols).transpose(1, 0, 2).reshape(128, 2 * cols)
    )


def _build_program(T):
    import concourse.bacc as bacc
    import concourse.mybir as mybir
    from concourse import tile

    f16 = mybir.dt.float16
    f32 = mybir.dt.float32
    Alu = mybir.AluOpType
    Act = mybir.ActivationFunctionType
    Ax = mybir.AxisListType

    nc = bacc.Bacc("TRN2", target_bir_lowering=False, debug=False, num_devices=NC)

    GR = ("b", "a")
    A_in = {g: nc.dram_tensor(f"A_{g}", [ACH, 128, 8 * SLOT], f16, kind="ExternalInput") for g in GR}
    H0_in = {g: nc.dram_tensor(f"h0_{g}", [2, 128, SLOT], f32, kind="ExternalInput") for g in GR}
    Wlin32_in = nc.dram_tensor("Wlin32", [128, 512], f32, kind="ExternalInput")
    Wlin16_in = nc.dram_tensor("Wlin16", [128, 512], f16, kind="ExternalInput")
    Wih16_in = nc.dram_tensor("Wih16", [128, 1536], f16, kind="ExternalInput")
    Whh16_in = nc.dram_tensor("Whh16", [128, 1536], f16, kind="ExternalInput")
    Wih32_in = nc.dram_tensor("Wih32", [128, 1536], f32, kind="ExternalInput")
    Whh32_in = nc.dram_tensor("Whh32", [128, 1536], f32, kind="ExternalInput")
    Blin_in = nc.dram_tensor("Blin", [128, 256], f32, kind="ExternalInput")
    Bcol_in = nc.dram_tensor("Bcol", [2, 128], f32, kind="ExternalInput")  # 1000*b_lin, feature-major
    Brz_in = nc.dram_tensor("Brz", [4, 128], f32, kind="ExternalInput")
    Bin_in = nc.dram_tensor("Bin", [2, 128], f32, kind="ExternalInput")
    Bhn_in = nc.dram_tensor("Bhn", [2, 128], f32, kind="ExternalInput")
    HO_out = {g: nc.dram_tensor(f"ho_{g}", [2, 128, SLOT], f32, kind="ExternalOutput") for g in GR}

    rg = [list(range(NC))]

    with tile.TileContext(nc) as tc:
        with (
            tc.tile_pool(name="const", bufs=1) as constp,
            tc.tile_pool(name="a_stream", bufs=2) as a_pool,
            tc.tile_pool(name="lhs_stream", bufs=4) as lhs_pool,
            tc.tile_pool(name="state", bufs=2) as state_pool,
            tc.tile_pool(name="state16", bufs=2) as state16_pool,
            tc.tile_pool(name="work", bufs=1) as work_pool,
            tc.tile_pool(name="tmp", bufs=2) as tmp_pool,
            tc.tile_pool(name="micro", bufs=2) as micro_pool,
            tc.tile_pool(name="psA", bufs=2, space="PSUM") as psum_agg,
            tc.tile_pool(name="psG", bufs=2, space="PSUM") as psum_gates,
            tc.tile_pool(name="dram", bufs=2, space="DRAM") as dram_pool,
        ):
            # ---- constants ----
            wlin32 = constp.tile([128, 512], f32, name="wlin32")
            nc.sync.dma_start(wlin32[:], Wlin32_in[:])
            wlin16 = constp.tile([128, 512], f16, name="wlin16")
            nc.sync.dma_start(wlin16[:], Wlin16_in[:])
            wih16 = constp.tile([128, 1536], f16, name="wih16")
            nc.sync.dma_start(wih16[:], Wih16_in[:])
            whh16 = constp.tile([128, 1536], f16, name="whh16")
            nc.sync.dma_start(whh16[:], Whh16_in[:])
            wih32 = constp.tile([128, 1536], f32, name="wih32")
            nc.sync.dma_start(wih32[:], Wih32_in[:])
            whh32 = constp.tile([128, 1536], f32, name="whh32")
            nc.sync.dma_start(whh32[:], Whh32_in[:])
            blin = constp.tile([128, 256], f32, name="blin")
            nc.sync.dma_start(blin[:], Blin_in[:])
            bcol = constp.tile([128, 2], f32, name="bcol")
            brz = constp.tile([128, 4], f32, name="brz")
            for i in range(4):
                nc.sync.dma_start(brz[:, i : i + 1], Brz_in[i : i + 1, :].rearrange("o p -> p o"))
            bin_ = constp.tile([128, 2], f32, name="bin_")
            bhn = constp.tile([128, 2], f32, name="bhn")
            for i in range(2):
                nc.sync.dma_start(bcol[:, i : i + 1], Bcol_in[i : i + 1, :].rearrange("o p -> p o"))
                nc.sync.dma_start(bin_[:, i : i + 1], Bin_in[i : i + 1, :].rearrange("o p -> p o"))
                nc.sync.dma_start(bhn[:, i : i + 1], Bhn_in[i : i + 1, :].rearrange("o p -> p o"))

            # ---- state load ----
            H = {}
            H16 = {}
            for g in GR:
                H[g] = []
                H16[g] = []
                for i in range(2):
                    h = state_pool.tile([128, SLOT], f32, name=f"h_{g}{i}", tag=f"h_{g}{i}")
                    nc.sync.dma_start(h[:], H0_in[g][i, :, :])
                    H[g].append(h)
                    h16 = state16_pool.tile([128, SLOT], f16, name=f"h16_{g}{i}", tag=f"h16_{g}{i}")
                    nc.vector.tensor_copy(h16[:], h[:])
                    H16[g].append(h16)

            cc_out = {}

            def emit_msgs_allgather(g):
                """fp16 msgs slice + fp32 colsum hi/lo -> cc_in -> AllGather."""
                msgs = work_pool.tile([128, 8 * 256], f16, name=f"msgs_{g}", tag=f"msgs_{g}")
                for mi in range(8):
                    ps = psum_gates.tile([128, 1024], f32, name=f"psm_{g}{mi}", tag="psG")
                    for kt in range(2):
                        nc.tensor.matmul(
                            ps[:, 0:256],
                            lhsT=H16[g][kt][:, mi * 128 : (mi + 1) * 128],
                            rhs=wlin16[:, kt * 256 : (kt + 1) * 256],
                            start=(kt == 0),
                            stop=(kt == 1),
                        )
                    nc.vector.tensor_add(msgs[:, mi * 256 : (mi + 1) * 256], ps[:, 0:256], blin[:])
                # exact supernode contribution: fp32 colsum of this core's msgs
                ps_cs = psum_gates.tile([128, 1024], f32, name=f"pscs_{g}", tag="psG")
                hs = []
                for kt in range(2):
                    hst = micro_pool.tile([128, 1], f32, name=f"hs_{g}{kt}", tag=f"hs_{g}{kt}")
                    nc.vector.tensor_reduce(hst[:], H[g][kt][:, 0:REAL], Ax.X, Alu.add)
                    hs.append(hst)
                for mi in range(2):
                    for kt in range(2):
                        nc.tensor.matmul(
                            ps_cs[:, mi : mi + 1],
                            lhsT=wlin32[:, kt * 256 + mi * 128 : kt * 256 + (mi + 1) * 128],
                            rhs=hs[kt][:],
                            start=(kt == 0),
                            stop=(kt == 1),
                        )
                cs_hi, cs_lo = [], []
                for mi in range(2):
                    cs = micro_pool.tile([128, 1], f32, name=f"cs_{g}{mi}", tag=f"cs_{g}{mi}")
                    nc.vector.tensor_scalar_add(cs[:], ps_cs[:, mi : mi + 1], bcol[:, mi : mi + 1])
                    hi = micro_pool.tile([128, 1], f16, name=f"cshi_{g}{mi}", tag=f"cshi_{g}{mi}")
                    nc.vector.tensor_copy(hi[:], cs[:])
                    lo = micro_pool.tile([128, 1], f16, name=f"cslo_{g}{mi}", tag=f"cslo_{g}{mi}")
                    nc.vector.tensor_sub(lo[:], cs[:], hi[:])
                    cs_hi.append(hi)
                    cs_lo.append(lo)
                cc_in = dram_pool.tile([SLOT, 256], f16, name=f"cc_in_{g}", tag=f"cc_in_{g}")
                nc.sync.dma_start(
                    cc_in[:].rearrange("(a p) f -> p a f", p=128),
                    msgs[:].rearrange("p (a f) -> p a f", a=8),
                )
                for mi in range(2):
                    nc.sync.dma_start(
                        cc_in[REAL : REAL + 1, mi * 128 : (mi + 1) * 128].rearrange("o f -> f o"),
                        cs_hi[mi][:],
                    )
                    nc.sync.dma_start(
                        cc_in[REAL + 1 : REAL + 2, mi * 128 : (mi + 1) * 128].rearrange("o f -> f o"),
                        cs_lo[mi][:],
                    )
                cco = dram_pool.tile(
                    [JTOT, 256], f16, name=f"cc_out_{g}", tag=f"cc_out_{g}", addr_space="Shared"
                )
                nc.gpsimd.collective_compute(
                    "AllGather",
                    mybir.AluOpType.bypass,
                    replica_groups=rg,
                    ins=[cc_in.opt()],
                    outs=[cco.opt()],
                )
                cc_out[g] = cco

            def emit_agg(g):
                """m.T [256, SLOT] = msgs_full.T @ A_shard.T via 64 k-tiles."""
                psA = [
                    psum_agg.tile([128, SLOT], f32, name=f"psA_{g}{mi}", tag="psA")
                    for mi in range(2)
                ]
                lhs_tiles = {}
                for q in range(NKT // 4):  # 16 lhs loads of 4 k-tiles each
                    off = 512 * q
                    lt = lhs_pool.tile([128, 1024], f16, name=f"lhs_{g}{q}", tag="lhs")
                    nc.sync.dma_start(
                        lt[:].rearrange("p (a f) -> p a f", a=4),
                        cc_out[g][off : off + 512, :].rearrange("(a p) f -> p a f", p=128),
                    )
                    lhs_tiles[q] = lt
                for g8 in range(ACH):
                    at = a_pool.tile([128, 8 * SLOT], f16, name=f"at_{g}{g8}", tag="at")
                    nc.sync.dma_start(at[:], A_in[g][g8, :, :])
                    for ktl in range(8):
                        kt = g8 * 8 + ktl
                        lt = lhs_tiles[kt // 4]
                        lo = (kt % 4) * 256
                        for mi in range(2):
                            for ni in range(2):
                                rhs_sl = at[:, ktl * SLOT + ni * 512 : ktl * SLOT + (ni + 1) * 512]
                                nc.tensor.matmul(
                                    psA[mi][:, ni * 512 : (ni + 1) * 512],
                                    lhsT=lt[:, lo + mi * 128 : lo + (mi + 1) * 128],
                                    rhs=rhs_sl,
                                    start=(kt == 0),
                                    stop=(kt == NKT - 1),
                                )
                m16 = []
                m_sup = []
                for mi in range(2):
                    msup = micro_pool.tile([128, 1], f32, name=f"msup_{g}{mi}", tag=f"msup_{g}{mi}")
                    nc.vector.tensor_copy(msup[:], psA[mi][:, REAL : REAL + 1])
                    m_sup.append(msup)
                    mt = work_pool.tile([128, SLOT], f16, name=f"m16_{g}{mi}", tag=f"m16_{g}{mi}")
                    nc.scalar.activation(mt[:], psA[mi][:], Act.Copy)
                    m16.append(mt)
                return m16, m_sup

            def emit_gru(g, m16, m_sup):
                """Gate matmuls (fp16) + fp32 elementwise GRU update of H[g]/H16[g].

                The supernode lane (column REAL) is recomputed in fp32 and
                overwritten into the new state tiles.
                """
                old_H = list(H[g])
                old_H16 = list(H16[g])

                def gate_psum(G, name):
                    ps = psum_gates.tile([128, 1024], f32, name=name, tag="psG")
                    for ni in range(2):
                        n_mm = 0
                        for kt in range(2):
                            for w, r in ((wih16, m16), (whh16, old_H16)):
                                nc.tensor.matmul(
                                    ps[:, ni * 512 : (ni + 1) * 512],
                                    lhsT=w[:, kt * 768 + G * 128 : kt * 768 + (G + 1) * 128],
                                    rhs=r[kt][:, ni * 512 : (ni + 1) * 512],
                                    start=(n_mm == 0),
                                    stop=(n_mm == 3),
                                )
                                n_mm += 1
                    return ps

                def half_psum(G, w, r, name):
                    ps = psum_gates.tile([128, 1024], f32, name=name, tag="psG")
                    for ni in range(2):
                        for kt in range(2):
                            nc.tensor.matmul(
                                ps[:, ni * 512 : (ni + 1) * 512],
                                lhsT=w[:, kt * 768 + G * 128 : kt * 768 + (G + 1) * 128],
                                rhs=r[kt][:, ni * 512 : (ni + 1) * 512],
                                start=(kt == 0),
                                stop=(kt == 1),
                            )
                    return ps

                # fp32 supernode gate psums: one psG slot, 8 columns
                # cols 0..3 = r0,r1,z0,z1 (gi+gh); 4,5 = inn0,inn1; 6,7 = hn0,hn1
                ps_s = psum_gates.tile([128, 1024], f32, name=f"ps_s{g}", tag="psG")
                h_sup = [old_H[kt][:, REAL : REAL + 1] for kt in range(2)]
                for G in range(4):
                    n_mm = 0
                    for kt in range(2):
                        for w, r in ((wih32, m_sup), (whh32, h_sup)):
                            nc.tensor.matmul(
                                ps_s[:, G : G + 1],
                                lhsT=w[:, kt * 768 + G * 128 : kt * 768 + (G + 1) * 128],
                                rhs=r[kt],
                                start=(n_mm == 0),
                                stop=(n_mm == 3),
                            )
                            n_mm += 1
                for ch in range(2):
                    for col, w, r in ((4 + ch, wih32, m_sup), (6 + ch, whh32, h_sup)):
                        for kt in range(2):
                            nc.tensor.matmul(
                                ps_s[:, col : col + 1],
                                lhsT=w[:, kt * 768 + (4 + ch) * 128 : kt * 768 + (5 + ch) * 128],
                                rhs=r[kt],
                                start=(kt == 0),
                                stop=(kt == 1),
                            )

                rr, zz = [], []
                for ch in range(2):
                    ps = gate_psum(ch, f"ps_r{g}{ch}")
                    r_t = work_pool.tile([128, SLOT], f32, name=f"r_{g}{ch}", tag=f"r_{g}{ch}")
                    nc.scalar.activation(r_t[:], ps[:], Act.Sigmoid, bias=brz[:, ch : ch + 1])
                    rr.append(r_t)
                for ch in range(2):
                    ps = gate_psum(2 + ch, f"ps_z{g}{ch}")
                    z_t = work_pool.tile([128, SLOT], f32, name=f"z_{g}{ch}", tag=f"z_{g}{ch}")
                    nc.scalar.activation(z_t[:], ps[:], Act.Sigmoid, bias=brz[:, 2 + ch : 3 + ch])
                    zz.append(z_t)

                # supernode fp32 lane: r/z/n + update into [128,1] tiles
                sup_new = []
                for ch in range(2):
                    rs = micro_pool.tile([128, 1], f32, name=f"rs_{g}{ch}", tag=f"rs_{g}{ch}")
                    nc.scalar.activation(rs[:], ps_s[:, ch : ch + 1], Act.Sigmoid, bias=brz[:, ch : ch + 1])
                    zs = micro_pool.tile([128, 1], f32, name=f"zs_{g}{ch}", tag=f"zs_{g}{ch}")
                    nc.scalar.activation(zs[:], ps_s[:, 2 + ch : 3 + ch], Act.Sigmoid, bias=brz[:, 2 + ch : 3 + ch])
                    t1s = micro_pool.tile([128, 1], f32, name=f"t1s_{g}{ch}", tag=f"t1s_{g}{ch}")
                    nc.vector.scalar_tensor_tensor(
                        t1s[:], ps_s[:, 6 + ch : 7 + ch], bhn[:, ch : ch + 1], rs[:], Alu.add, Alu.mult
                    )
                    t2s = micro_pool.tile([128, 1], f32, name=f"t2s_{g}{ch}", tag=f"t2s_{g}{ch}")
                    nc.vector.tensor_add(t2s[:], t1s[:], ps_s[:, 4 + ch : 5 + ch])
                    ns = micro_pool.tile([128, 1], f32, name=f"ns_{g}{ch}", tag=f"ns_{g}{ch}")
                    nc.scalar.activation(ns[:], t2s[:], Act.Tanh, bias=bin_[:, ch : ch + 1])
                    ds = micro_pool.tile([128, 1], f32, name=f"ds_{g}{ch}", tag=f"ds_{g}{ch}")
                    nc.vector.tensor_sub(ds[:], h_sup[ch], ns[:])
                    t3s = micro_pool.tile([128, 1], f32, name=f"t3s_{g}{ch}", tag=f"t3s_{g}{ch}")
                    nc.vector.tensor_mul(t3s[:], zs[:], ds[:])
                    hns = micro_pool.tile([128, 1], f32, name=f"hns_{g}{ch}", tag=f"hns_{g}{ch}")
                    nc.vector.tensor_add(hns[:], ns[:], t3s[:])
                    sup_new.append(hns)

                for ch in range(2):
                    ps_i = half_psum(4 + ch, wih16, m16, f"ps_i{g}{ch}")
                    ps_h = half_psum(4 + ch, whh16, old_H16, f"ps_h{g}{ch}")
                    t1 = tmp_pool.tile([128, SLOT], f32, name=f"t1_{g}{ch}", tag=f"tmp_{g}")
                    nc.vector.scalar_tensor_tensor(
                        t1[:], ps_h[:], bhn[:, ch : ch + 1], rr[ch][:], Alu.add, Alu.mult
                    )
                    t2 = tmp_pool.tile([128, SLOT], f32, name=f"t2_{g}{ch}", tag=f"tmp_{g}")
                    nc.vector.tensor_add(t2[:], t1[:], ps_i[:])
                    n_t = tmp_pool.tile([128, SLOT], f32, name=f"n_{g}{ch}", tag=f"n_{g}")
                    nc.scalar.activation(n_t[:], t2[:], Act.Tanh, bias=bin_[:, ch : ch + 1])
                    d_t = tmp_pool.tile([128, SLOT], f32, name=f"d_{g}{ch}", tag=f"tmp_{g}")
                    nc.vector.tensor_sub(d_t[:], old_H[ch][:], n_t[:])
                    t3 = tmp_pool.tile([128, SLOT], f32, name=f"t3_{g}{ch}", tag=f"tmp_{g}")
                    nc.vector.tensor_mul(t3[:], zz[ch][:], d_t[:])
                    hn_new = state_pool.tile([128, SLOT], f32, name=f"h_{g}{ch}", tag=f"h_{g}{ch}")
                    nc.vector.tensor_add(hn_new[:], n_t[:], t3[:])
                    # overwrite supernode lane with the fp32 value
                    nc.vector.tensor_copy(hn_new[:, REAL : REAL + 1], sup_new[ch][:])
                    H[g][ch] = hn_new
                    h16_new = state16_pool.tile(
                        [128, SLOT], f16, name=f"h16_{g}{ch}", tag=f"h16_{g}{ch}"
                    )
                    nc.scalar.activation(h16_new[:], hn_new[:], Act.Copy)
                    H16[g][ch] = h16_new

            if T >= 1:
                for g in GR:
                    emit_msgs_allgather(g)
                for t in range(T):
                    for g in GR:
                        m16, m_sup = emit_agg(g)
                        emit_gru(g, m16, m_sup)
                        if t < T - 1:
                            emit_msgs_allgather(g)

            for g in GR:
                for i in range(2):
                    nc.sync.dma_start(HO_out[g][i, :, :], H[g][i][:])

    nc.compile()
    return nc


def prepare(inputs):
    """Build+compile the program and the per-core input maps.

    Returns (nc, in_maps, postprocess) where postprocess maps core 7's
    result dict to the final [2] log-softmax output.
    """
    b_x = np.asarray(inputs["b_x"], dtype=np.float32)
    a_x = np.asarray(inputs["a_x"], dtype=np.float32)
    b_adj = np.asarray(inputs["b_adj"], dtype=np.float32)
    a_adj = np.asarray(inputs["a_adj"], dtype=np.float32)
    W_lin = np.asarray(inputs["W_lin"], dtype=np.float32)
    b_lin = np.asarray(inputs["b_lin"], dtype=np.float32)
    W_ih = np.asarray(inputs["W_ih"], dtype=np.float32)
    b_ih = np.asarray(inputs["b_ih"], dtype=np.float32)
    W_hh = np.asarray(inputs["W_hh"], dtype=np.float32)
    b_hh = np.asarray(inputs["b_hh"], dtype=np.float32)
    W_fc = np.asarray(inputs["W_fc"], dtype=np.float32)
    b_fc = np.asarray(inputs["b_fc"], dtype=np.float32)
    T = int(inputs["n_timesteps"])

    nc = _build_program(T)

    A_shards = {"b": _prep_adj_shards(b_adj), "a": _prep_adj_shards(a_adj)}
    H0_shards = {"b": _prep_h0_shards(b_x), "a": _prep_h0_shards(a_x)}
    wlin32_p = _pack_lhsT(W_lin.T, 256, np.float32)
    wlin16_p = _pack_lhsT(W_lin.T, 256, np.float16)
    wih16_p = _pack_lhsT(W_ih.T, 768, np.float16)
    whh16_p = _pack_lhsT(W_hh.T, 768, np.float16)
    wih32_p = _pack_lhsT(W_ih.T, 768, np.float32)
    whh32_p = _pack_lhsT(W_hh.T, 768, np.float32)
    blin_b = np.ascontiguousarray(np.broadcast_to(b_lin.astype(np.float32), (128, 256)))
    bcol_p = np.ascontiguousarray((float(REAL) * b_lin).astype(np.float32).reshape(2, 128))
    brz_p = np.ascontiguousarray((b_ih[:512] + b_hh[:512]).astype(np.float32).reshape(4, 128))
    bin_p = np.ascontiguousarray(b_ih[512:768].astype(np.float32).reshape(2, 128))
    bhn_p = np.ascontiguousarray(b_hh[512:768].astype(np.float32).reshape(2, 128))

    in_maps = []
    for c in range(NC):
        in_maps.append(
            {
                "A_b": A_shards["b"][c],
                "A_a": A_shards["a"][c],
                "h0_b": H0_shards["b"][c],
                "h0_a": H0_shards["a"][c],
                "Wlin32": wlin32_p,
                "Wlin16": wlin16_p,
                "Wih16": wih16_p,
                "Whh16": whh16_p,
                "Wih32": wih32_p,
                "Whh32": whh32_p,
                "Blin": blin_b,
                "Bcol": bcol_p,
                "Brz": brz_p,
                "Bin": bin_p,
                "Bhn": bhn_p,
            }
        )

    def post(out7):
        sup = np.concatenate(
            [
                np.asarray(out7["ho_b"]).reshape(HIDDEN, SLOT)[:, REAL],
                np.asarray(out7["ho_a"]).reshape(HIDDEN, SLOT)[:, REAL],
            ]
        ).astype(np.float64)
        logits = sup @ W_fc.astype(np.float64).T + b_fc.astype(np.float64)
        mx = logits.max()
        return (logits - mx - np.log(np.exp(logits - mx).sum())).astype(np.float32)

    return nc, in_maps, post


def run(inputs, trace=False):
    from concourse.bass_utils import run_bass_kernel_spmd

    nc, in_maps, post = prepare(inputs)
    res = run_bass_kernel_spmd(nc, in_maps, core_ids=list(range(NC)), trace=trace)
    return post(res.results[NC - 1]), res.exec_time_ns


def kernel(**inputs):
    out, _ = run(inputs, trace=False)
    return out


# revision 4
# speedup vs baseline: 1.4044x; 1.0628x over previous
"""GGNN (JITGNN) Trainium2 kernel v2: 8-core row-parallel SpMM message passing.

Strategy (per sharding hint): shard the [N+1, N+1] adjacency row-wise across
8 cores. Each core keeps the h-state for its 1000(+1) nodes in transposed
(feature-major) layout in SBUF, computes its slice of messages each timestep,
AllGathers fp16 messages across cores, then streams its pre-transposed
adjacency shard as the matmul moving operand to aggregate, and applies the
GRU cell to its slice. Two independent graphs (b, a) are interleaved so each
graph's collective hides behind the other graph's compute.

Numerics: fp16 matmul operands (messages, adjacency, gate weights) with fp32
PSUM accumulation and fp32 state/elementwise. The final output depends only
on the supernode row, which sums ~8000 messages; the supernode gets an exact
path: each core computes its fp32 message column-sum (via fp32 row-reduced h
and an fp32 matmul), transmits it as an fp16 hi+lo pair through the same
AllGather (rows 1000/1001 of its shard block, which are otherwise padding),
and the supernode's GRU lane is recomputed in fp32. Final 2-class head on
host in fp64.
"""

import numpy as np

try:
    import concourse.bacc  # noqa: F401
except ImportError:  # pragma: no cover
    import sys

    sys.path.insert(0, "/opt/trn_rl_repo")

F16 = np.float16
HIDDEN = 256
N = 8000
NC = 8             # cores
SLOT = 1024        # padded node slots per core (1000 real, +1 supernode on core 7)
REAL = N // NC     # 1000 real rows per core
JTOT = NC * SLOT   # 8192 padded message rows
NKT = JTOT // 128  # 64 contraction k-tiles
ACH = 8            # A chunks per graph, each [128, 8192] = 8 k-tiles x 1024 cols


def _prep_adj_shards(adj):
    """adj [8000,8000] 0/1 fp32 -> per-core rhs chunks [ACH, 128, 8192] fp16.

    R_c[j', u] = A_aug[i(u), j(j')] with j' = 1024*d + r (msgs row layout of
    the AllGather output), u = local output slot. The supernode column
    (u=1000 on core 7) sums the per-core fp32 colsum hi/lo rows at
    j' = 1024*d + {1000, 1001} instead of the raw message rows.
    """
    AT = np.ascontiguousarray(adj.T.astype(F16))             # [j, i]
    ATj = np.zeros((JTOT, N), dtype=F16)
    for d in range(NC):
        ATj[SLOT * d : SLOT * d + REAL] = AT[REAL * d : REAL * (d + 1)]
    supersum = np.zeros((JTOT,), dtype=F16)
    for d in range(NC):
        supersum[SLOT * d + REAL] = 1.0      # colsum hi row
        supersum[SLOT * d + REAL + 1] = 1.0  # colsum lo row
    shards = []
    for c in range(NC):
        R = np.zeros((JTOT, SLOT), dtype=F16)
        R[:, :REAL] = ATj[:, REAL * c : REAL * (c + 1)]
        if c == NC - 1:
            R[:, REAL] = supersum
        chunks = R.reshape(ACH, 8, 128, SLOT).transpose(0, 2, 1, 3).reshape(ACH, 128, 8 * SLOT)
        shards.append(np.ascontiguousarray(chunks))
    return shards


def _prep_h0_shards(x):
    """x [8000, 256] fp32 -> per-core transposed state [2, 128, SLOT] fp32."""
    xT = x.T.astype(np.float32)  # [256, 8000]
    shards = []
    for c in range(NC):
        H = np.zeros((HIDDEN, SLOT), dtype=np.float32)
        H[:, :REAL] = xT[:, REAL * c : REAL * (c + 1)]
        shards.append(np.ascontiguousarray(H.reshape(2, 128, SLOT)))
    return shards


def _pack_lhsT(w_t, cols, dt):
    """w_t [256, cols] -> packed [128, 2*cols] with free = kt*cols + c."""
    return np.ascontiguousarray(
        w_t.astype(dt).reshape(2, 128, cols).transpose(1, 0, 2).reshape(128, 2 * cols)
    )


def _build_program(T, zero_blin=False):
    import concourse.bacc as bacc
    import concourse.mybir as mybir
    from concourse import tile

    f16 = mybir.dt.float16
    f32 = mybir.dt.float32
    Alu = mybir.AluOpType
    Act = mybir.ActivationFunctionType
    Ax = mybir.AxisListType

    nc = bacc.Bacc("TRN2", target_bir_lowering=False, debug=False, num_devices=NC)

    GR = ("b", "a")
    A_in = {g: nc.dram_tensor(f"A_{g}", [ACH, 128, 8 * SLOT], f16, kind="ExternalInput") for g in GR}
    H0_in = {g: nc.dram_tensor(f"h0_{g}", [2, 128, SLOT], f32, kind="ExternalInput") for g in GR}
    Wlin32_in = nc.dram_tensor("Wlin32", [128, 512], f32, kind="ExternalInput")
    Wlin16_in = nc.dram_tensor("Wlin16", [128, 512], f16, kind="ExternalInput")
    Wih16_in = nc.dram_tensor("Wih16", [128, 1536], f16, kind="ExternalInput")
    Whh16_in = nc.dram_tensor("Whh16", [128, 1536], f16, kind="ExternalInput")
    Wih32_in = nc.dram_tensor("Wih32", [128, 1536], f32, kind="ExternalInput")
    Whh32_in = nc.dram_tensor("Whh32", [128, 1536], f32, kind="ExternalInput")
    Blin_in = nc.dram_tensor("Blin", [128, 256], f32, kind="ExternalInput")
    Bcol_in = nc.dram_tensor("Bcol", [2, 128], f32, kind="ExternalInput")  # 1000*b_lin, feature-major
    Brz_in = nc.dram_tensor("Brz", [4, 128], f32, kind="ExternalInput")
    Bin_in = nc.dram_tensor("Bin", [2, 128], f32, kind="ExternalInput")
    Bhn_in = nc.dram_tensor("Bhn", [2, 128], f32, kind="ExternalInput")
    HO_out = {g: nc.dram_tensor(f"ho_{g}", [2, 128, SLOT], f32, kind="ExternalOutput") for g in GR}

    rg = [list(range(NC))]

    with tile.TileContext(nc) as tc:
        with (
            tc.tile_pool(name="const", bufs=1) as constp,
            tc.tile_pool(name="a_stream", bufs=2) as a_pool,
            tc.tile_pool(name="lhs_stream", bufs=4) as lhs_pool,
            tc.tile_pool(name="state", bufs=2) as state_pool,
            tc.tile_pool(name="state16", bufs=2) as state16_pool,
            tc.tile_pool(name="work", bufs=1) as work_pool,
            tc.tile_pool(name="tmp", bufs=2) as tmp_pool,
            tc.tile_pool(name="micro", bufs=2) as micro_pool,
            tc.tile_pool(name="psA", bufs=2, space="PSUM") as psum_agg,
            tc.tile_pool(name="psG", bufs=2, space="PSUM") as psum_gates,
            tc.tile_pool(name="dram", bufs=2, space="DRAM") as dram_pool,
        ):
            # ---- constants ----
            wlin32 = constp.tile([128, 512], f32, name="wlin32")
            nc.sync.dma_start(wlin32[:], Wlin32_in[:])
            wlin16 = constp.tile([128, 512], f16, name="wlin16")
            nc.sync.dma_start(wlin16[:], Wlin16_in[:])
            wih16 = constp.tile([128, 1536], f16, name="wih16")
            nc.sync.dma_start(wih16[:], Wih16_in[:])
            whh16 = constp.tile([128, 1536], f16, name="whh16")
            nc.sync.dma_start(whh16[:], Whh16_in[:])
            wih32 = constp.tile([128, 1536], f32, name="wih32")
            nc.sync.dma_start(wih32[:], Wih32_in[:])
            whh32 = constp.tile([128, 1536], f32, name="whh32")
            nc.sync.dma_start(whh32[:], Whh32_in[:])
            blin = constp.tile([128, 256], f32, name="blin")
            nc.sync.dma_start(blin[:], Blin_in[:])
            bcol = constp.tile([128, 2], f32, name="bcol")
            brz = constp.tile([128, 4], f32, name="brz")
            for i in range(4):
                nc.sync.dma_start(brz[:, i : i + 1], Brz_in[i : i + 1, :].rearrange("o p -> p o"))
            bin_ = constp.tile([128, 2], f32, name="bin_")
            bhn = constp.tile([128, 2], f32, name="bhn")
            for i in range(2):
                nc.sync.dma_start(bcol[:, i : i + 1], Bcol_in[i : i + 1, :].rearrange("o p -> p o"))
                nc.sync.dma_start(bin_[:, i : i + 1], Bin_in[i : i + 1, :].rearrange("o p -> p o"))
                nc.sync.dma_start(bhn[:, i : i + 1], Bhn_in[i : i + 1, :].rearrange("o p -> p o"))

            # ---- state load ----
            H = {}
            H16 = {}
            for g in GR:
                H[g] = []
                H16[g] = []
                for i in range(2):
                    h = state_pool.tile([128, SLOT], f32, name=f"h_{g}{i}", tag=f"h_{g}{i}")
                    nc.sync.dma_start(h[:], H0_in[g][i, :, :])
                    H[g].append(h)
                    h16 = state16_pool.tile([128, SLOT], f16, name=f"h16_{g}{i}", tag=f"h16_{g}{i}")
                    nc.vector.tensor_copy(h16[:], h[:])
                    H16[g].append(h16)

            cc_out = {}

            def emit_msgs_allgather(g):
                """fp16 msgs slice + fp32 colsum hi/lo -> cc_in -> AllGather."""
                msgs = work_pool.tile([128, 8 * 256], f16, name=f"msgs_{g}", tag=f"msgs_{g}")
                for mi in range(8):
                    ps = psum_gates.tile([128, 1024], f32, name=f"psm_{g}{mi}", tag="psG")
                    for kt in range(2):
                        nc.tensor.matmul(
                            ps[:, 0:256],
                            lhsT=H16[g][kt][:, mi * 128 : (mi + 1) * 128],
                            rhs=wlin16[:, kt * 256 : (kt + 1) * 256],
                            start=(kt == 0),
                            stop=(kt == 1),
                        )
                    if zero_blin:
                        nc.scalar.activation(msgs[:, mi * 256 : (mi + 1) * 256], ps[:, 0:256], Act.Copy)
                    else:
                        nc.vector.tensor_add(msgs[:, mi * 256 : (mi + 1) * 256], ps[:, 0:256], blin[:])
                # exact supernode contribution: fp32 colsum of this core's msgs
                ps_cs = psum_gates.tile([128, 1024], f32, name=f"pscs_{g}", tag="psG")
                hs = []
                for kt in range(2):
                    hst = micro_pool.tile([128, 1], f32, name=f"hs_{g}{kt}", tag=f"hs_{g}{kt}")
                    nc.vector.tensor_reduce(hst[:], H[g][kt][:, 0:REAL], Ax.X, Alu.add)
                    hs.append(hst)
                for mi in range(2):
                    for kt in range(2):
                        nc.tensor.matmul(
                            ps_cs[:, mi : mi + 1],
                            lhsT=wlin32[:, kt * 256 + mi * 128 : kt * 256 + (mi + 1) * 128],
                            rhs=hs[kt][:],
                            start=(kt == 0),
                            stop=(kt == 1),
                        )
                cs_hi, cs_lo = [], []
                for mi in range(2):
                    cs = micro_pool.tile([128, 1], f32, name=f"cs_{g}{mi}", tag=f"cs_{g}{mi}")
                    if zero_blin:
                        nc.vector.tensor_copy(cs[:], ps_cs[:, mi : mi + 1])
                    else:
                        nc.vector.tensor_scalar_add(cs[:], ps_cs[:, mi : mi + 1], bcol[:, mi : mi + 1])
                    hi = micro_pool.tile([128, 1], f16, name=f"cshi_{g}{mi}", tag=f"cshi_{g}{mi}")
                    nc.vector.tensor_copy(hi[:], cs[:])
                    lo = micro_pool.tile([128, 1], f16, name=f"cslo_{g}{mi}", tag=f"cslo_{g}{mi}")
                    nc.vector.tensor_sub(lo[:], cs[:], hi[:])
                    cs_hi.append(hi)
                    cs_lo.append(lo)
                cc_in = dram_pool.tile([SLOT, 256], f16, name=f"cc_in_{g}", tag=f"cc_in_{g}")
                nc.sync.dma_start(
                    cc_in[:].rearrange("(a p) f -> p a f", p=128),
                    msgs[:].rearrange("p (a f) -> p a f", a=8),
                )
                for mi in range(2):
                    nc.sync.dma_start(
                        cc_in[REAL : REAL + 1, mi * 128 : (mi + 1) * 128].rearrange("o f -> f o"),
                        cs_hi[mi][:],
                    )
                    nc.sync.dma_start(
                        cc_in[REAL + 1 : REAL + 2, mi * 128 : (mi + 1) * 128].rearrange("o f -> f o"),
                        cs_lo[mi][:],
                    )
                cco = dram_pool.tile(
                    [JTOT, 256], f16, name=f"cc_out_{g}", tag=f"cc_out_{g}", addr_space="Shared"
                )
                nc.gpsimd.collective_compute(
                    "AllGather",
                    mybir.AluOpType.bypass,
                    replica_groups=rg,
                    ins=[cc_in.opt()],
                    outs=[cco.opt()],
                )
                cc_out[g] = cco

            def emit_agg(g):
                """m.T [256, SLOT] = msgs_full.T @ A_shard.T via 64 k-tiles."""
                psA = [
                    psum_agg.tile([128, SLOT], f32, name=f"psA_{g}{mi}", tag="psA")
                    for mi in range(2)
                ]
                lhs_tiles = {}
                for q in range(NKT // 4):  # 16 lhs loads of 4 k-tiles each
                    off = 512 * q
                    lt = lhs_pool.tile([128, 1024], f16, name=f"lhs_{g}{q}", tag="lhs")
                    nc.sync.dma_start(
                        lt[:].rearrange("p (a f) -> p a f", a=4),
                        cc_out[g][off : off + 512, :].rearrange("(a p) f -> p a f", p=128),
                    )
                    lhs_tiles[q] = lt
                for g8 in range(ACH):
                    at = a_pool.tile([128, 8 * SLOT], f16, name=f"at_{g}{g8}", tag="at")
                    nc.sync.dma_start(at[:], A_in[g][g8, :, :])
                    for ktl in range(8):
                        kt = g8 * 8 + ktl
                        lt = lhs_tiles[kt // 4]
                        lo = (kt % 4) * 256
                        for mi in range(2):
                            for ni in range(2):
                                rhs_sl = at[:, ktl * SLOT + ni * 512 : ktl * SLOT + (ni + 1) * 512]
                                nc.tensor.matmul(
                                    psA[mi][:, ni * 512 : (ni + 1) * 512],
                                    lhsT=lt[:, lo + mi * 128 : lo + (mi + 1) * 128],
                                    rhs=rhs_sl,
                                    start=(kt == 0),
                                    stop=(kt == NKT - 1),
                                )
                m16 = []
                m_sup = []
                for mi in range(2):
                    msup = micro_pool.tile([128, 1], f32, name=f"msup_{g}{mi}", tag=f"msup_{g}{mi}")
                    nc.vector.tensor_copy(msup[:], psA[mi][:, REAL : REAL + 1])
                    m_sup.append(msup)
                    mt = work_pool.tile([128, SLOT], f16, name=f"m16_{g}{mi}", tag=f"m16_{g}{mi}")
                    nc.scalar.activation(mt[:], psA[mi][:], Act.Copy)
                    m16.append(mt)
                return m16, m_sup

            def emit_gru(g, m16, m_sup):
                """Gate matmuls (fp16) + fp32 elementwise GRU update of H[g]/H16[g].

                The supernode lane (column REAL) is recomputed in fp32 and
                overwritten into the new state tiles.
                """
                old_H = list(H[g])
                old_H16 = list(H16[g])

                def gate_psum(G, name):
                    ps = psum_gates.tile([128, 1024], f32, name=name, tag="psG")
                    for ni in range(2):
                        n_mm = 0
                        for kt in range(2):
                            for w, r in ((wih16, m16), (whh16, old_H16)):
                                nc.tensor.matmul(
                                    ps[:, ni * 512 : (ni + 1) * 512],
                                    lhsT=w[:, kt * 768 + G * 128 : kt * 768 + (G + 1) * 128],
                                    rhs=r[kt][:, ni * 512 : (ni + 1) * 512],
                                    start=(n_mm == 0),
                                    stop=(n_mm == 3),
                                )
                                n_mm += 1
                    return ps

                def half_psum(G, w, r, name):
                    ps = psum_gates.tile([128, 1024], f32, name=name, tag="psG")
                    for ni in range(2):
                        for kt in range(2):
                            nc.tensor.matmul(
                                ps[:, ni * 512 : (ni + 1) * 512],
                                lhsT=w[:, kt * 768 + G * 128 : kt * 768 + (G + 1) * 128],
                                rhs=r[kt][:, ni * 512 : (ni + 1) * 512],
                                start=(kt == 0),
                                stop=(kt == 1),
                            )
                    return ps

                # fp32 supernode gate psums: one psG slot, 8 columns
                # cols 0..3 = r0,r1,z0,z1 (gi+gh); 4,5 = inn0,inn1; 6,7 = hn0,hn1
                ps_s = psum_gates.tile([128, 1024], f32, name=f"ps_s{g}", tag="psG")
                h_sup = [old_H[kt][:, REAL : REAL + 1] for kt in range(2)]
                for G in range(4):
                    n_mm = 0
                    for kt in range(2):
                        for w, r in ((wih32, m_sup), (whh32, h_sup)):
                            nc.tensor.matmul(
                                ps_s[:, G : G + 1],
                                lhsT=w[:, kt * 768 + G * 128 : kt * 768 + (G + 1) * 128],
                                rhs=r[kt],
                                start=(n_mm == 0),
                                stop=(n_mm == 3),
                            )
                            n_mm += 1
                for ch in range(2):
                    for col, w, r in ((4 + ch, wih32, m_sup), (6 + ch, whh32, h_sup)):
                        for kt in range(2):
                            nc.tensor.matmul(
                                ps_s[:, col : col + 1],
                                lhsT=w[:, kt * 768 + (4 + ch) * 128 : kt * 768 + (5 + ch) * 128],
                                rhs=r[kt],
                                start=(kt == 0),
                                stop=(kt == 1),
                            )

                rr, zz = [], []
                for ch in range(2):
                    ps = gate_psum(ch, f"ps_r{g}{ch}")
                    r_t = work_pool.tile([128, SLOT], f32, name=f"r_{g}{ch}", tag=f"r_{g}{ch}")
                    nc.scalar.activation(r_t[:], ps[:], Act.Sigmoid, bias=brz[:, ch : ch + 1])
                    rr.append(r_t)
                for ch in range(2):
                    ps = gate_psum(2 + ch, f"ps_z{g}{ch}")
                    z_t = work_pool.tile([128, SLOT], f32, name=f"z_{g}{ch}", tag=f"z_{g}{ch}")
                    nc.scalar.activation(z_t[:], ps[:], Act.Sigmoid, bias=brz[:, 2 + ch : 3 + ch])
                    zz.append(z_t)

                # supernode fp32 lane: r/z/n + update into [128,1] tiles
                sup_new = []
                for ch in range(2):
                    rs = micro_pool.tile([128, 1], f32, name=f"rs_{g}{ch}", tag=f"rs_{g}{ch}")
                    nc.scalar.activation(rs[:], ps_s[:, ch : ch + 1], Act.Sigmoid, bias=brz[:, ch : ch + 1])
                    zs = micro_pool.tile([128, 1], f32, name=f"zs_{g}{ch}", tag=f"zs_{g}{ch}")
                    nc.scalar.activation(zs[:], ps_s[:, 2 + ch : 3 + ch], Act.Sigmoid, bias=brz[:, 2 + ch : 3 + ch])
                    t1s = micro_pool.tile([128, 1], f32, name=f"t1s_{g}{ch}", tag=f"t1s_{g}{ch}")
                    nc.vector.scalar_tensor_tensor(
                        t1s[:], ps_s[:, 6 + ch : 7 + ch], bhn[:, ch : ch + 1], rs[:], Alu.add, Alu.mult
                    )
                    t2s = micro_pool.tile([128, 1], f32, name=f"t2s_{g}{ch}", tag=f"t2s_{g}{ch}")
                    nc.vector.tensor_add(t2s[:], t1s[:], ps_s[:, 4 + ch : 5 + ch])
                    ns = micro_pool.tile([128, 1], f32, name=f"ns_{g}{ch}", tag=f"ns_{g}{ch}")
                    nc.scalar.activation(ns[:], t2s[:], Act.Tanh, bias=bin_[:, ch : ch + 1])
                    ds = micro_pool.tile([128, 1], f32, name=f"ds_{g}{ch}", tag=f"ds_{g}{ch}")
                    nc.vector.tensor_sub(ds[:], h_sup[ch], ns[:])
                    t3s = micro_pool.tile([128, 1], f32, name=f"t3s_{g}{ch}", tag=f"t3s_{g}{ch}")
                    nc.vector.tensor_mul(t3s[:], zs[:], ds[:])
                    hns = micro_pool.tile([128, 1], f32, name=f"hns_{g}{ch}", tag=f"hns_{g}{ch}")
                    nc.vector.tensor_add(hns[:], ns[:], t3s[:])
                    sup_new.append(hns)

                for ch in range(2):
                    ps_i = half_psum(4 + ch, wih16, m16, f"ps_i{g}{ch}")
                    ps_h = half_psum(4 + ch, whh16, old_H16, f"ps_h{g}{ch}")
                    t1 = tmp_pool.tile([128, SLOT], f32, name=f"t1_{g}{ch}", tag=f"tmp_{g}")
                    nc.vector.scalar_tensor_tensor(
                        t1[:], ps_h[:], bhn[:, ch : ch + 1], rr[ch][:], Alu.add, Alu.mult
                    )
                    t2 = tmp_pool.tile([128, SLOT], f32, name=f"t2_{g}{ch}", tag=f"tmp_{g}")
                    nc.vector.tensor_add(t2[:], t1[:], ps_i[:])
                    n_t = tmp_pool.tile([128, SLOT], f32, name=f"n_{g}{ch}", tag=f"n_{g}")
                    nc.scalar.activation(n_t[:], t2[:], Act.Tanh, bias=bin_[:, ch : ch + 1])
                    d_t = tmp_pool.tile([128, SLOT], f32, name=f"d_{g}{ch}", tag=f"tmp_{g}")
                    nc.vector.tensor_sub(d_t[:], old_H[ch][:], n_t[:])
                    t3 = tmp_pool.tile([128, SLOT], f32, name=f"t3_{g}{ch}", tag=f"tmp_{g}")
                    nc.vector.tensor_mul(t3[:], zz[ch][:], d_t[:])
                    hn_new = state_pool.tile([128, SLOT], f32, name=f"h_{g}{ch}", tag=f"h_{g}{ch}")
                    nc.vector.tensor_add(hn_new[:], n_t[:], t3[:])
                    # overwrite supernode lane with the fp32 value
                    nc.vector.tensor_copy(hn_new[:, REAL : REAL + 1], sup_new[ch][:])
                    H[g][ch] = hn_new
                    h16_new = state16_pool.tile(
                        [128, SLOT], f16, name=f"h16_{g}{ch}", tag=f"h16_{g}{ch}"
                    )
                    nc.scalar.activation(h16_new[:], hn_new[:], Act.Copy)
                    H16[g][ch] = h16_new

            if T >= 1:
                for g in GR:
                    emit_msgs_allgather(g)
                for t in range(T):
                    for g in GR:
                        m16, m_sup = emit_agg(g)
                        emit_gru(g, m16, m_sup)
                        if t < T - 1:
                            emit_msgs_allgather(g)

            for g in GR:
                for i in range(2):
                    nc.sync.dma_start(HO_out[g][i, :, :], H[g][i][:])

    nc.compile()
    return nc


def prepare(inputs):
    """Build+compile the program and the per-core input maps.

    Returns (nc, in_maps, postprocess) where postprocess maps core 7's
    result dict to the final [2] log-softmax output.
    """
    b_x = np.asarray(inputs["b_x"], dtype=np.float32)
    a_x = np.asarray(inputs["a_x"], dtype=np.float32)
    b_adj = np.asarray(inputs["b_adj"], dtype=np.float32)
    a_adj = np.asarray(inputs["a_adj"], dtype=np.float32)
    W_lin = np.asarray(inputs["W_lin"], dtype=np.float32)
    b_lin = np.asarray(inputs["b_lin"], dtype=np.float32)
    W_ih = np.asarray(inputs["W_ih"], dtype=np.float32)
    b_ih = np.asarray(inputs["b_ih"], dtype=np.float32)
    W_hh = np.asarray(inputs["W_hh"], dtype=np.float32)
    b_hh = np.asarray(inputs["b_hh"], dtype=np.float32)
    W_fc = np.asarray(inputs["W_fc"], dtype=np.float32)
    b_fc = np.asarray(inputs["b_fc"], dtype=np.float32)
    T = int(inputs["n_timesteps"])

    nc = _build_program(T, zero_blin=not np.any(b_lin))

    A_shards = {"b": _prep_adj_shards(b_adj), "a": _prep_adj_shards(a_adj)}
    H0_shards = {"b": _prep_h0_shards(b_x), "a": _prep_h0_shards(a_x)}
    wlin32_p = _pack_lhsT(W_lin.T, 256, np.float32)
    wlin16_p = _pack_lhsT(W_lin.T, 256, np.float16)
    wih16_p = _pack_lhsT(W_ih.T, 768, np.float16)
    whh16_p = _pack_lhsT(W_hh.T, 768, np.float16)
    wih32_p = _pack_lhsT(W_ih.T, 768, np.float32)
    whh32_p = _pack_lhsT(W_hh.T, 768, np.float32)
    blin_b = np.ascontiguousarray(np.broadcast_to(b_lin.astype(np.float32), (128, 256)))
    bcol_p = np.ascontiguousarray((float(REAL) * b_lin).astype(np.float32).reshape(2, 128))
    brz_p = np.ascontiguousarray((b_ih[:512] + b_hh[:512]).astype(np.float32).reshape(4, 128))
    bin_p = np.ascontiguousarray(b_ih[512:768].astype(np.float32).reshape(2, 128))
    bhn_p = np.ascontiguousarray(b_hh[512:768].astype(np.float32).reshape(2, 128))

    in_maps = []
    for c in range(NC):
        in_maps.append(
            {
                "A_b": A_shards["b"][c],
                "A_a": A_shards["a"][c],
                "h0_b": H0_shards["b"][c],
                "h0_a": H0_shards["a"][c],
                "Wlin32": wlin32_p,
                "Wlin16": wlin16_p,
                "Wih16": wih16_p,
                "Whh16": whh16_p,
                "Wih32": wih32_p,
                "Whh32": whh32_p,
                "Blin": blin_b,
                "Bcol": bcol_p,
                "Brz": brz_p,
                "Bin": bin_p,
                "Bhn": bhn_p,
            }
        )

    def post(out7):
        sup = np.concatenate(
            [
                np.asarray(out7["ho_b"]).reshape(HIDDEN, SLOT)[:, REAL],
                np.asarray(out7["ho_a"]).reshape(HIDDEN, SLOT)[:, REAL],
            ]
        ).astype(np.float64)
        logits = sup @ W_fc.astype(np.float64).T + b_fc.astype(np.float64)
        mx = logits.max()
        return (logits - mx - np.log(np.exp(logits - mx).sum())).astype(np.float32)

    return nc, in_maps, post


def run(inputs, trace=False):
    from concourse.bass_utils import run_bass_kernel_spmd

    nc, in_maps, post = prepare(inputs)
    res = run_bass_kernel_spmd(nc, in_maps, core_ids=list(range(NC)), trace=trace)
    return post(res.results[NC - 1]), res.exec_time_ns


def kernel(**inputs):
    out, _ = run(inputs, trace=False)
    return out
